# revision 26
# baseline (speedup 1.0000x reference)
"""BnBinActiveConv2d Trainium2 kernel.

Pipeline (per reference):
  BN (batch stats, train mode) -> BinActive (sign + K = box(mean_c |xn|))
  -> BinConv (sign weights) -> relu(y * K * alpha)

Math used by this kernel (gamma > 0 assumed; gamma==1, beta==0 in practice):
  s_c = gamma_c * rsqrt(var_c + eps),  t_c = mu_c - beta_c / s_c
  sign(xn) = sign(x - t_c)
  |xn|     = |s_c * x - s_c * t_c|
  out      = relu(alpha_co * y) * K,  K folded as raw box-sum with the
             1/(9C) normalization folded into alpha.

Sharding: data-parallel over batch, 4 images per core on 8 cores. Only the
BN statistics (mean, E[x^2] per channel: 2KB) are all-reduced across cores.

The conv is 9 shifted fp8e4m3 DoubleRow matmuls per output tile (one per
tap, contracting all 256 input channels as 128 partitions x 2) accumulated
in PSUM; sign values are exact in fp8 so sums are exact integers in fp32
PSUM. The rhs streams flat full-width row chunks (56 cols incl. 2 junk
cols per row, dropped at PSUM evacuation) to keep the DoubleRow AP 3D.

v2 vs baseline (per-exec ~200us -> ~100us measured, session-dependent):
  - |xn| in fp8 and the A channel-sum as one DoubleRow matmul per chunk
  - K, the K broadcast, the y scale multiply and the y output in bf16
    (host casts back to fp32); halves that DMA traffic and doubles the
    DVE multiply rate
  - alpha computed on-device from wt via DoubleRow matmuls (no wo input)
  - x loads queued ahead of weight loads; weight prep overlaps the stats
    exchange window
  - BN stats exchanged via AllGather + local DVE reduce (cheaper and
    better-hidden than the ncfw AllReduce, which measured ~47us serial)
  - BN rsqrt via one Sqrt activation + reciprocal + one Newton step,
    batched over both channel halves (one activation table set total)
  - box filter batched over image pairs on 108 partitions
  - queue topology: x loads + A-path smalls on the sync HWDGE ring,
    K broadcast + AllGather on the gpsimd queue, stats staging on the
    scalar ring, y stores on sync; chosen from HW ablations (x loads on
    SWDGE, y stores on the scalar ring, relu on DVE all measured worse)
  - sign/sign/abs/abs activation order (conv gates on signs only) and
    8 y_t buffers so PSUM evacuation never waits on the K broadcast
"""

import numpy as np
from contextlib import ExitStack

import concourse.bass as bass
import concourse.bacc as bacc
import concourse.tile as tile
from concourse import mybir
from concourse.alu_op_type import AluOpType

AFT = mybir.ActivationFunctionType
FP32 = mybir.dt.float32
BF16 = mybir.dt.bfloat16
FP8 = mybir.dt.float8e4

BN_EPS = 1e-4
P = 128


def _chunk(total, cap):
    """Largest divisor of `total` that is <= cap."""
    for c in range(min(cap, total), 0, -1):
        if total % c == 0:
            return c
    return total


def build(B_loc=4, C=256, H=56, KS=3, n_cores=8, loop_k=1, ablate=()):
    W_ = H
    HO, WO = H - KS + 1, W_ - KS + 1
    CC = C // P
    assert CC == 2
    HW, HOWO = H * W_, HO * WO
    CKK = C * KS * KS

    GF = _chunk(HO, 512 // W_)         # full-W rows per conv psum tile
    NGF = HO // GF
    HWP = HW + 64                      # padded per-chunk stride for fp8 xs/ad
    NJ = _chunk(HW, 448)               # A-sum / bn_stats subgroup
    JN = HW // NJ
    DR = mybir.MatmulPerfMode.DoubleRow

    nc = bacc.Bacc("TRN2", target_bir_lowering=False, debug=False,
                   enable_asserts=False, num_devices=n_cores)

    x_d = nc.dram_tensor("x", [B_loc, C, H, W_], FP32, kind="ExternalInput").ap()
    wt_d = nc.dram_tensor("wt", [C, KS, KS, C], FP32, kind="ExternalInput").ap()
    g_d = nc.dram_tensor("gamma", [C], FP32, kind="ExternalInput").ap()
    b_d = nc.dram_tensor("beta", [C], FP32, kind="ExternalInput").ap()
    y_d = nc.dram_tensor("y", [B_loc, C, HO, WO], BF16, kind="ExternalOutput").ap()

    with tile.TileContext(nc) as tc:
        with ExitStack() as ctx:
            consts = ctx.enter_context(tc.tile_pool(name="consts", bufs=1))
            statsp = ctx.enter_context(tc.tile_pool(name="stats", bufs=1))
            wbp = ctx.enter_context(tc.tile_pool(name="wb", bufs=1))
            dram = ctx.enter_context(tc.tile_pool(name="dram", bufs=1, space="DRAM"))

            xpool = ctx.enter_context(tc.tile_pool(
                name="x", bufs=9 if "x10" in ablate else 1))
            xsp = ctx.enter_context(tc.tile_pool(name="xs", bufs=4))
            adp = ctx.enter_context(tc.tile_pool(name="ad", bufs=2))
            arp = ctx.enter_context(tc.tile_pool(name="arow", bufs=2))
            boxp = ctx.enter_context(tc.tile_pool(name="box", bufs=2))
            kbcp = ctx.enter_context(tc.tile_pool(name="kbc", bufs=2))
            yp = ctx.enter_context(tc.tile_pool(
                name="y", bufs=4 if "yp4" in ablate else 8))
            psA = ctx.enter_context(tc.tile_pool(name="psA", bufs=2, space="PSUM"))
            psC = ctx.enter_context(tc.tile_pool(name="psC", bufs=4, space="PSUM"))

            ones8_t = consts.tile([P, CC, 16], FP8)
            nc.vector.memset(ones8_t, 1.0)
            ones8 = ones8_t[:, :, 0:1]   # Ko stride 16 (dual-fp8 ISA rule)
            gam = consts.tile([P, CC], FP32)
            nc.scalar.dma_start(out=gam, in_=g_d.rearrange("(cc p) -> p cc", p=P))
            bet = consts.tile([P, CC], FP32)
            nc.scalar.dma_start(out=bet, in_=b_d.rearrange("(cc p) -> p cc", p=P))

            alpha_s = consts.tile([P, CC], FP32)   # alpha/(C*KS^2)^2, per co
            neg_t = consts.tile([P, CC], FP32)     # -t_c per ci
            nst = consts.tile([P, CC], FP32)       # -s_c * t_c per ci
            s_sb = consts.tile([P, CC], FP32)      # s_c per ci
            eps_sb = statsp.tile([P, 1], FP32)
            nc.vector.memset(eps_sb, BN_EPS)

            wb8 = wbp.tile([P, CC, KS * KS * C], FP8, name="wb8")
            araw_d = dram.tile([B_loc, HW], BF16)
            kflat_d = dram.tile([B_loc, HOWO], BF16)
            cc_in = dram.tile([P, 2 * CC], FP32)
            cc_out = dram.tile([P, 2 * CC], FP32)
            cc_ag = dram.tile([n_cores, P, 2 * CC], FP32)

            for rep in range(loop_k):
                # ---- phase 1: load x, per-core BN partial stats ----
                x_sb = {}
                stats = [statsp.tile([P, B_loc * JN, 6], FP32, tag=f"st{cc}",
                                     name=f"st{cc}r{rep}")
                         for cc in range(CC)]
                for n in range(B_loc):
                    for cc in range(CC):
                        xt = xpool.tile([P, HW], FP32,
                                        tag="x" if "x10" in ablate else f"x{n}{cc}",
                                        name=f"xt{n}{cc}r{rep}")
                        # SWDGE queue: keeps rep n+1 x loads off the sync
                        # ring so they aren't FIFO-blocked behind rep n's
                        # y stores (cross-rep overlap)
                        x_eng = nc.sync if "gpsx" not in ablate else nc.gpsimd
                        x_eng.dma_start(
                            out=xt,
                            in_=x_d[n, cc * P:(cc + 1) * P].rearrange("c h w -> c (h w)"))
                        x_sb[n, cc] = xt
                        xr = xt.rearrange("p (j v) -> p j v", v=NJ)
                        for j in range(JN):
                            nc.vector.bn_stats(out=stats[cc][:, n * JN + j, :],
                                               in_=xr[:, j, :])

                if rep == 0:
                    # ---- weights: queued behind x on the sync ring, so the
                    # prep overlaps the stats all-reduce window ----
                    aw8 = wbp.tile([P, CC, KS * KS * C], FP8, name="aw8")
                    KH = KS * KS * C // 2
                    with tc.tile_pool(name="wtmp", bufs=2) as wtmp:
                        for cc in range(CC):
                            wv = wt_d[cc * P:(cc + 1) * P].rearrange(
                                "c a b o -> c (a b o)")
                            for h in range(2):
                                wt_f = wtmp.tile([P, KH], FP32, tag="wtmp")
                                nc.sync.dma_start(
                                    out=wt_f, in_=wv[:, h * KH:(h + 1) * KH])
                                nc.scalar.activation(
                                    out=wb8[:, cc, h * KH:(h + 1) * KH],
                                    in_=wt_f, func=AFT.Sign)
                                nc.scalar.activation(
                                    out=aw8[:, cc, h * KH:(h + 1) * KH],
                                    in_=wt_f, func=AFT.Abs)
                    # alpha[co] = sum_{ci,kh,kw} |W| via DoubleRow matmuls
                    aw8v = aw8.rearrange("p c (k o) -> p c k o", o=C)
                    for coh in range(CC):
                        pal = psA.tile([P, 1], FP32, tag="psA", name=f"pal{coh}")
                        for k in range(KS * KS):
                            nc.tensor.matmul(
                                pal,
                                lhsT=aw8v[:, :, k, coh * P:(coh + 1) * P],
                                rhs=ones8,
                                start=(k == 0), stop=(k == KS * KS - 1),
                                perf_mode=DR)
                        # fold alpha 1/CKK and the K box 1/(C*KS^2) = 1/CKK
                        nc.vector.tensor_scalar_mul(
                            alpha_s[:, coh:coh + 1], pal, 1.0 / (CKK * CKK))

                # ---- phase 2: all-reduce stats, derive s, t ----
                pk = statsp.tile([P, 2 * CC], FP32, tag="pk", name=f"pk{rep}")
                for cc in range(CC):
                    mv = statsp.tile([P, 2], FP32, tag="mv")
                    nc.vector.bn_aggr(out=mv, in_=stats[cc])
                    nc.vector.tensor_copy(out=pk[:, 2 * cc:2 * cc + 1], in_=mv[:, 0:1])
                    # m2 = mean^2 + var
                    nc.vector.scalar_tensor_tensor(
                        out=pk[:, 2 * cc + 1:2 * cc + 2], in0=mv[:, 0:1],
                        scalar=mv[:, 0:1], in1=mv[:, 1:2],
                        op0=AluOpType.mult, op1=AluOpType.add)
                nc.scalar.dma_start(out=cc_in, in_=pk)
                sums = statsp.tile([P, 2 * CC], FP32, tag="sums", name=f"sums{rep}")
                if "arred" in ablate:
                    nc.gpsimd.collective_compute(
                        "AllReduce", AluOpType.add,
                        replica_groups=[list(range(n_cores))],
                        ins=[cc_in.opt()], outs=[cc_out.opt()])
                    nc.scalar.dma_start(out=sums, in_=cc_out)
                elif "ar" in ablate:
                    nc.scalar.dma_start(out=cc_out, in_=cc_in)
                    nc.scalar.dma_start(out=sums, in_=cc_out)
                else:
                    # AllGather (≈2x cheaper than AllReduce) + local reduce
                    nc.gpsimd.collective_compute(
                        "AllGather", AluOpType.bypass,
                        replica_groups=[list(range(n_cores))],
                        ins=[cc_in.opt()], outs=[cc_ag.opt()])
                    srt = statsp.tile([P, n_cores, 2 * CC], FP32, tag="srt",
                                      name=f"srt{rep}")
                    nc.scalar.dma_start(
                        out=srt,
                        in_=bass.AP(tensor=cc_ag.tensor, offset=cc_ag.offset,
                                    ap=[[2 * CC, P], [P * 2 * CC, n_cores],
                                        [1, 2 * CC]]))
                    sv8 = bass.AP(tensor=srt.tensor, offset=srt.offset,
                                  ap=list(srt.ap[:1]) + [[1, 2 * CC],
                                                         [2 * CC, n_cores]])
                    nc.vector.tensor_reduce(out=sums, in_=sv8,
                                            axis=mybir.AxisListType.X,
                                            op=AluOpType.add)
                nc.vector.tensor_scalar_mul(
                    sums, sums, 1.0 / (n_cores if "ar" not in ablate else 1))

                sv = sums.rearrange("p (c two) -> p c two", two=2)
                mean = sv[:, :, 0]
                ex2 = sv[:, :, 1]
                var = statsp.tile([P, CC], FP32, tag="var")
                m2 = statsp.tile([P, CC], FP32, tag="m2")
                nc.vector.tensor_tensor(out=m2, in0=mean, in1=mean, op=AluOpType.mult)
                nc.vector.tensor_tensor(out=var, in0=ex2, in1=m2,
                                        op=AluOpType.subtract)
                r = statsp.tile([P, CC], FP32, tag="r")
                nc.scalar.activation(out=r, in_=var, func=AFT.Sqrt, bias=eps_sb)
                nc.vector.reciprocal(out=r, in_=r)
                # one Newton step: r <- r * (1.5 - 0.5 * (var+eps) * r^2)
                ve = statsp.tile([P, CC], FP32, tag="ve")
                nc.vector.tensor_scalar(out=ve, in0=var, scalar1=1.0, scalar2=BN_EPS,
                                        op0=AluOpType.mult, op1=AluOpType.add)
                rr = statsp.tile([P, CC], FP32, tag="rr")
                nc.vector.tensor_tensor(out=rr, in0=r, in1=r, op=AluOpType.mult)
                nc.vector.tensor_tensor(out=rr, in0=rr, in1=ve, op=AluOpType.mult)
                nc.vector.tensor_scalar(out=rr, in0=rr, scalar1=-0.5, scalar2=1.5,
                                        op0=AluOpType.mult, op1=AluOpType.add)
                nc.vector.tensor_tensor(out=r, in0=r, in1=rr, op=AluOpType.mult)
                nc.vector.tensor_tensor(out=s_sb, in0=r, in1=gam, op=AluOpType.mult)
                inv_s = statsp.tile([P, CC], FP32, tag="invs")
                nc.vector.reciprocal(out=inv_s, in_=s_sb)
                # neg_t = beta * (1/s) - mean
                nc.vector.tensor_tensor(out=inv_s, in0=bet, in1=inv_s,
                                        op=AluOpType.mult)
                nc.vector.tensor_tensor(out=neg_t, in0=inv_s, in1=mean,
                                        op=AluOpType.subtract)
                nc.vector.tensor_tensor(out=nst, in0=s_sb, in1=neg_t,
                                        op=AluOpType.mult)

                # ---- phase 3a: per image binarize, A-sum, box K, broadcast ----
                xs8_sb, kbc_sb = {}, {}
                for n in range(B_loc):
                    xs8 = xsp.tile([P, CC, HWP], FP8, tag="xs", name=f"xs8{n}r{rep}")
                    nc.vector.memset(xs8[:, :, HW:HWP], 0.0)
                    xs8_sb[n] = xs8
                    ad8 = adp.tile([P, CC, HWP], FP8, tag="ad", name=f"ad8{n}r{rep}")
                    if "sign" not in ablate:
                        order = ([(0, 0), (1, 0), (0, 1), (1, 1)]
                                 if "oldord" in ablate
                                 else [(0, 0), (0, 1), (1, 0), (1, 1)])
                        for which, cc in order:
                            if which == 0:
                                nc.scalar.activation(out=xs8[:, cc, 0:HW],
                                                     in_=x_sb[n, cc],
                                                     func=AFT.Sign,
                                                     bias=neg_t[:, cc:cc + 1])
                            else:
                                nc.scalar.activation(out=ad8[:, cc, 0:HW],
                                                     in_=x_sb[n, cc],
                                                     func=AFT.Abs,
                                                     bias=nst[:, cc:cc + 1],
                                                     scale=s_sb[:, cc:cc + 1])
                    else:
                        for cc in range(CC):
                            nc.vector.memset(xs8[:, cc, 0:HW], 1.0)
                            nc.vector.memset(ad8[:, cc, 0:HW], 1.0)

                    # A raw channel-sum: ones.T @ |xn| (fp8 DoubleRow) -> [1, HW]
                    a_row = arp.tile([1, HW], BF16, tag="arow")
                    for j in range(JN):
                        pa = psA.tile([1, NJ], FP32, tag="psA")
                        nc.tensor.matmul(
                            pa, lhsT=ones8,
                            rhs=ad8[:, :, j * NJ:(j + 1) * NJ],
                            start=True, stop=True, perf_mode=DR)
                        nc.vector.tensor_copy(out=a_row[:, j * NJ:(j + 1) * NJ],
                                              in_=pa)
                    sm_eng = nc.gpsimd if "smallpool" in ablate else nc.sync
                    sm_eng.dma_start(out=araw_d[n:n + 1, :], in_=a_row)

                    # box filter: paired (108 partitions, half the DMA
                    # chains on one queue) or per-image ('unpair': earlier K
                    # for the pipeline head)
                    unpair = "unpair" in ablate
                    if not unpair and n % 2 == 0:
                        continue
                    NB = 1 if unpair else 2
                    n0 = n if unpair else n - 1
                    a_sh = boxp.tile([NB * HO, KS, W_], BF16, tag="ash")
                    sm_eng.dma_start(
                        out=a_sh,
                        in_=bass.AP(
                            tensor=araw_d.tensor,
                            offset=araw_d.offset + n0 * HW,
                            ap=[[HW, NB], [W_, HO], [W_, KS], [1, W_]]))
                    t1 = boxp.tile([NB * HO, W_], FP32, tag="t1")
                    nc.vector.tensor_tensor(out=t1, in0=a_sh[:, 0, :],
                                            in1=a_sh[:, 1, :], op=AluOpType.add)
                    nc.vector.tensor_tensor(out=t1, in0=t1, in1=a_sh[:, 2, :],
                                            op=AluOpType.add)
                    k_im = boxp.tile([NB * HO, WO], BF16, tag="kim")
                    t2 = boxp.tile([NB * HO, WO], FP32, tag="t2")
                    nc.vector.tensor_tensor(out=t2, in0=t1[:, 0:WO],
                                            in1=t1[:, 1:WO + 1], op=AluOpType.add)
                    nc.vector.tensor_tensor(out=k_im, in0=t2, in1=t1[:, 2:WO + 2],
                                            op=AluOpType.add)
                    sm_eng.dma_start(
                        out=bass.AP(
                            tensor=kflat_d.tensor,
                            offset=kflat_d.offset + n0 * HOWO,
                            ap=[[WO, NB * HO], [1, WO]]),
                        in_=k_im)
                    for m in range(n0, n + 1):
                        k_bc = kbcp.tile([P, HOWO], BF16, tag="kbc",
                                         name=f"kbc{m}r{rep}")
                        ksrc = kflat_d[m, :]
                        nc.gpsimd.dma_start(
                            out=k_bc,
                            in_=bass.AP(tensor=ksrc.tensor, offset=ksrc.offset,
                                        ap=[[0, P]] + list(ksrc.ap)))
                        kbc_sb[m] = k_bc

                # ---- phase 3b: conv + scale + store ----
                wb8v = wb8.rearrange("p c (k o) -> p c k o", o=C)
                for n in (() if "conv" in ablate else range(B_loc)):
                    xs8 = xs8_sb[n]
                    for co in range(CC):
                        for g in range(NGF):
                            pc = psC.tile([P, GF, W_], FP32, tag="psC")
                            first = True
                            for kh in range(KS):
                                for kw in range(KS):
                                    last = (kh == KS - 1 and kw == KS - 1)
                                    off = g * GF * W_ + kh * W_ + kw
                                    nc.tensor.matmul(
                                        pc,
                                        lhsT=wb8v[:, :, kh * KS + kw,
                                                  co * P:(co + 1) * P],
                                        rhs=xs8[:, :, off:off + GF * W_],
                                        start=first, stop=last, perf_mode=DR)
                                    first = False
                            y_t = yp.tile([P, GF * WO], BF16, tag="y")
                            if "dverelu" in ablate or ("relusplit" in ablate
                                                       and co == 1):
                                nc.vector.tensor_scalar(
                                    out=y_t.rearrange("p (g w) -> p g w", w=WO),
                                    in0=pc[:, :, 0:WO],
                                    scalar1=alpha_s[:, co:co + 1], scalar2=0.0,
                                    op0=AluOpType.mult, op1=AluOpType.max)
                            else:
                                nc.scalar.activation(
                                    out=y_t.rearrange("p (g w) -> p g w", w=WO),
                                    in_=pc[:, :, 0:WO],
                                    func=AFT.Relu, scale=alpha_s[:, co:co + 1])
                            if "kmul" not in ablate:
                                eng = nc.gpsimd if "poolmul" in ablate else nc.vector
                                eng.tensor_tensor(
                                    out=y_t, in0=y_t,
                                    in1=kbc_sb[n][:, g * GF * WO:(g + 1) * GF * WO],
                                    op=AluOpType.mult)
                            st_eng = nc.scalar if "actstore" in ablate else nc.sync
                            st_eng.dma_start(
                                out=y_d[n, co * P:(co + 1) * P,
                                        g * GF:(g + 1) * GF, :].rearrange(
                                            "c h w -> c (h w)"),
                                in_=y_t)

    nc.compile()
    return nc


_CACHE = {}


def _get_compiled():
    if "nc" not in _CACHE:
        _CACHE["nc"] = build()
    return _CACHE["nc"]


def make_in_maps(x, gamma, beta, W, n_cores=8):
    x = np.ascontiguousarray(np.asarray(x, dtype=np.float32))
    gamma = np.ascontiguousarray(np.asarray(gamma, dtype=np.float32))
    beta = np.ascontiguousarray(np.asarray(beta, dtype=np.float32))
    W = np.asarray(W, dtype=np.float32)
    wt = np.ascontiguousarray(np.transpose(W, (1, 2, 3, 0)))
    B_loc = x.shape[0] // n_cores
    return [
        {"x": np.ascontiguousarray(x[c * B_loc:(c + 1) * B_loc]),
         "wt": wt, "gamma": gamma, "beta": beta}
        for c in range(n_cores)
    ]


def run(x, gamma, beta, W, trace=False):
    from concourse import bass_utils
    nc = _get_compiled()
    in_maps = make_in_maps(x, gamma, beta, W)
    res = bass_utils.run_bass_kernel_spmd(nc, in_maps, core_ids=list(range(8)),
                                          trace=trace)
    out = np.concatenate([r["y"] for r in res.results], axis=0)
    return out.astype(np.float32), res


def kernel(x, gamma, beta, W):
    out, _ = run(x, gamma, beta, W)
    return out
